# revision 19
# baseline (speedup 1.0000x reference)
"""AlphaCompositor on 8 TRN2 NeuronCores.

Data-parallel over the view axis N (one image per core). The per-pixel
point-feature gather (1M random 16B rows per core) is the whole problem:
the stock ``indirect_dma_start`` tops out at 128 indices per ~1.4us call
(SWDGE fixed cost) -> 11.6ms baseline. Instead we use the MOE
``dma_gather`` ucode (InstDMAGatherAnt): one Pool instruction gathers
2048 indices (129 descs/lane, so 7 calls fit the 1024-slot SWDGE ring;
rotating across all 4 SWDGE queues keeps same-queue ring reclaim ~28
calls behind and removes the per-call stall; small calls also generate
descriptors at ~4ns/idx vs 7.8ns at 8192). Its int16 indices
can't span P=100000 rows, so the host
pads the table to 256B-strided *blocks* of 4 rows (int16 block index
< 25000) and each gather pulls the 64B block; a DVE select-from-4
(folded into the compositing weight) picks the right row.

Per core pipeline (depth 2):
  A_k: load plane k (natural layout for alpha math + a second copy in
       the ucode's 16-partition-wrapped order), compute masked alpha,
       sub-row id, and the int16 block indices on DVE.
  G_k: 32 dma_gather calls -> G[k%2] = [128, 512, 16] (4 rows/pixel).
  C_k: DVE compositing: w = a*t, t -= w, then for j in 0..3:
       acc += (sub==j)*w * G[:, :, 4j:4j+4]; plane-0 background fill.

The 64B-elem dma_gather bypasses a bass-level elem%256 assert that the
ucode does not actually require (only the row stride is encoded in 256B
units); the instruction is constructed directly.
"""

import os
import sys

sys.path.insert(0, "/opt/trn_rl_repo")

import numpy as np

N, K, H, W = 8, 16, 256, 256
C, P = 4, 100000
PIX = H * W  # 65536
PPART = 128
FREE = PIX // PPART  # 512

RPB = 4  # table rows per gather block
NBLK = P // RPB  # 25000 (< int16 max)
BLKF = 64  # floats per padded block (256B stride)
CAP = 4096  # indices per dma_gather call: ~16.4us/call is a fixed floor
# for CAP<=4096 (16KB idx scratch fits Q7 dcache; 8192 thrashes), so use
# the largest cache-friendly size; 257 descs/lane, 3 fit the ring, and
# 4-queue rotation keeps same-queue reclaim 12 calls behind
CALLS = PIX // CAP  # 8
SLOT = CAP // PPART  # 64 gathered columns per call
IW = CAP // 16  # 512 idx columns per call

_CACHE = {}


def _dma_gather_raw(gp, out_ap, in_ap, idxs_ap, num_idxs, elem_size, elem_step,
                    queue_num=0, single_packet=False):
    """BassGpSimd.dma_gather (non-transpose, HBM source) minus the
    elem_size%256 assert; the ucode only needs stride%256==0."""
    import concourse.mybir as mybir
    from concourse import ap_utils
    from concourse._compat import exact_div

    assert idxs_ap.tensor.dtype == mybir.dt.int16
    assert in_ap.dtype == out_ap.dtype
    assert in_ap.ap[0][0] == elem_step
    assert in_ap.ap[-1][1] == out_ap.ap[-1][1] == elem_size
    assert out_ap.ap[0][1] * out_ap.ap[1][1] == (num_idxs + 127) // 128 * 128
    assert ap_utils.ap_is_contiguous(out_ap.ap[1:])
    assert ap_utils.ap_is_contiguous(idxs_ap.ap[1:])
    stride_bytes = elem_step * mybir.dt.size(in_ap.dtype)
    stride_bytes_256 = exact_div(stride_bytes, 256)
    assert stride_bytes_256 < 256

    _in_ap = gp.lower_ap_dma(in_ap, for_custom_bir_dma=True)
    _idxs_ap = gp.lower_ap(idxs_ap)
    _out_ap = gp.lower_ap(out_ap)
    return gp.add_instruction(
        mybir.InstDMAGatherAnt(
            name=gp.bass.get_next_instruction_name(),
            ins=[*_in_ap, _idxs_ap, gp.lower_val_access(gp.to_reg(num_idxs))],
            outs=[_out_ap],
            transpose=False,
            num_idxs=num_idxs,
            elem_size=elem_size,
            stride_bytes_256=stride_bytes_256,
            gen_mode=0,
            single_packet=single_packet,
            queue_num=queue_num,
            sbuf_tokens_per_rank=0,
            sbuf_free_dim_per_rank=0,
            sbuf_free_dim_pad_per_rank=0,
            sbuf_byte_offset=0,
        )
    )


def _build_nc():
    import concourse.mybir as mybir
    import concourse.tile as tile
    from concourse import bacc, library_config

    f32 = mybir.dt.float32
    i32 = mybir.dt.int32
    i16 = mybir.dt.int16
    Alu = mybir.AluOpType

    nc = bacc.Bacc(None, target_bir_lowering=False, num_swdge_queues=4)
    frag_d = nc.declare_dram_parameter("frag", [K, PIX], i32, isOutput=False)
    fragw_d = nc.declare_dram_parameter("fragw", [K, 16, PIX // 16], i32,
                                        isOutput=False)
    alpha_d = nc.declare_dram_parameter("alpha", [K, PIX], f32, isOutput=False)
    tbl_d = nc.declare_dram_parameter("tbl", [NBLK, BLKF], f32, isOutput=False)
    bg_d = nc.declare_dram_parameter("bg", [1, C], f32, isOutput=False)
    out_d = nc.declare_dram_parameter("out", [C, PIX], f32, isOutput=True)

    tblv = tbl_d[:, 0 : RPB * C]  # [(64,25000),(1,16)] -> elem 16, step 64

    with tile.TileContext(nc) as tc:
        nc.gpsimd.load_library(library_config.mlp)
        with (
            tc.tile_pool(name="io", bufs=3) as io_pool,
            tc.tile_pool(name="gp", bufs=2) as g_pool,
            tc.tile_pool(name="persist", bufs=1) as pp,
        ):
            acc = pp.tile([PPART, FREE, C], f32)
            t = pp.tile([PPART, FREE], f32)
            m = pp.tile([PPART, FREE], f32)
            bg = pp.tile([PPART, 1, C], f32)
            nc.vector.memset(t[:], 1.0)
            nc.sync.dma_start(out=bg[:, 0, :], in_=bg_d[:, :].to_broadcast([PPART, C]))

            a_t, sub_t, idx_t = {}, {}, {}

            def phaseA(k):
                fk = io_pool.tile([PPART, FREE], i32, tag="frag")
                ak = io_pool.tile([PPART, FREE], f32, tag="alpha")
                nc.sync.dma_start(
                    out=fk[:], in_=frag_d[k].rearrange("(p f) -> p f", p=PPART)
                )
                nc.sync.dma_start(
                    out=ak[:], in_=alpha_d[k].rearrange("(p f) -> p f", p=PPART)
                )
                # a = (frag >= 0) * alpha ; sub = frag & 3 (garbage when
                # invalid -- weight is 0 there)
                a = io_pool.tile([PPART, FREE], f32, tag="a")
                nc.vector.scalar_tensor_tensor(
                    out=a[:], in0=fk[:], scalar=0, in1=ak[:],
                    op0=Alu.is_ge, op1=Alu.mult,
                )
                # sub = frag - 4*(frag>>2)  (mod/bitwise-and fail ISA checks)
                sub = io_pool.tile([PPART, FREE], i32, tag="sub")
                nc.vector.tensor_scalar(
                    out=sub[:], in0=fk[:], scalar1=2, scalar2=None,
                    op0=Alu.arith_shift_right,
                )
                nc.vector.scalar_tensor_tensor(
                    out=sub[:], in0=sub[:], scalar=-4, in1=fk[:],
                    op0=Alu.mult, op1=Alu.add,
                )
                if k == 0:
                    nc.vector.tensor_scalar(
                        out=m[:], in0=fk[:], scalar1=0, scalar2=None, op0=Alu.is_lt
                    )
                # wrapped copy -> int16 block indices, in two half-plane
                # chunks to bound SBUF (fw is i32 [128, 2048] per chunk)
                idx16 = io_pool.tile([PPART, CALLS * IW], i16, tag="idx16")
                half = CALLS * IW // 2  # 2048
                for h in range(2):
                    fw = io_pool.tile([PPART, half], i32, tag="fw")
                    src = fragw_d[k, :, h * half : (h + 1) * half]
                    nc.sync.dma_start(
                        out=fw[:],
                        in_=src.rearrange("q j -> () q j").to_broadcast(
                            [PPART // 16, 16, half]
                        ),
                    )
                    # block = max(frag, 0) >> 2, all-i32 (the TSP bitVec op
                    # can't cast or mix with arith), then cast-copy to int16
                    nc.vector.tensor_scalar_max(fw[:], fw[:], 0)
                    nc.vector.tensor_scalar(
                        out=fw[:], in0=fw[:], scalar1=2, scalar2=None,
                        op0=Alu.arith_shift_right,
                    )
                    nc.vector.tensor_copy(
                        out=idx16[:, h * half : (h + 1) * half], in_=fw[:]
                    )
                a_t[k], sub_t[k], idx_t[k] = a, sub, idx16

            def gathers(k):
                # alternate SWDGE queues so call N+1's Q7 descriptor
                # generation overlaps call N's SDMA drain (separate rings)
                G = g_pool.tile([PPART, FREE, RPB * C], f32, tag="G")
                for mm in range(CALLS):
                    _dma_gather_raw(
                        nc.gpsimd,
                        out_ap=G[:, mm * SLOT : (mm + 1) * SLOT, :],
                        in_ap=tblv,
                        idxs_ap=idx_t[k][:, mm * IW : (mm + 1) * IW],
                        num_idxs=CAP,
                        elem_size=RPB * C,
                        elem_step=BLKF,
                        queue_num=(k * CALLS + mm) % 4,
                    )
                return G

            def comp(k, G):
                w = io_pool.tile([PPART, FREE], f32, tag="w")
                nc.vector.tensor_tensor(out=w[:], in0=a_t[k][:], in1=t[:], op=Alu.mult)
                if k < K - 1:
                    nc.vector.tensor_tensor(out=t[:], in0=t[:], in1=w[:], op=Alu.subtract)
                if k == 0:
                    m3 = m[:].rearrange("p (f o) -> p f o", o=1).to_broadcast(
                        [PPART, FREE, C]
                    )
                    bg3 = bg[:].to_broadcast([PPART, FREE, C])
                    nc.vector.tensor_tensor(out=acc[:], in0=m3, in1=bg3, op=Alu.mult)
                for j in range(RPB):
                    mj = io_pool.tile([PPART, FREE], f32, tag="mj")
                    nc.vector.tensor_scalar(
                        out=mj[:], in0=sub_t[k][:], scalar1=j, scalar2=None,
                        op0=Alu.is_equal,
                    )
                    nc.vector.tensor_tensor(out=mj[:], in0=mj[:], in1=w[:], op=Alu.mult)
                    wj3 = mj[:].rearrange("p (f o) -> p f o", o=1).to_broadcast(
                        [PPART, FREE, C]
                    )
                    gj = G[:, :, j * C : (j + 1) * C]
                    tmp = io_pool.tile([PPART, FREE, C], f32, tag="tmp")
                    nc.vector.tensor_tensor(out=tmp[:], in0=gj, in1=wj3, op=Alu.mult)
                    nc.vector.tensor_tensor(out=acc[:], in0=acc[:], in1=tmp[:], op=Alu.add)

            phaseA(0)
            phaseA(1)
            Gs = {}
            for k in range(K):
                if k + 2 < K:
                    phaseA(k + 2)
                Gs[k] = gathers(k)
                comp(k, Gs[k])

            for c in range(C):
                pl = io_pool.tile([PPART, FREE], f32, tag="pl")
                nc.scalar.copy(out=pl[:], in_=acc[:, :, c])
                nc.sync.dma_start(
                    out=out_d[c].rearrange("(p f) -> p f", p=PPART), in_=pl[:]
                )

    nc.compile()
    return nc


def _get_nc():
    if "nc" not in _CACHE:
        _CACHE["nc"] = _build_nc()
    return _CACHE["nc"]


# wrapped-order permutation for the dma_gather index stream: pixel at
# natural tile position (p, s) is logical token i of its plane; the ucode
# reads token i from partition i%16, column i//16 of each 8192-index call.
def _wrap_perm():
    i = np.arange(PIX)
    mcall = i // CAP
    l = i % CAP
    q = l % 16
    jg = mcall * IW + l // 16
    p = l % PPART
    s = mcall * SLOT + l // PPART
    x = p * FREE + s  # natural flat pixel id
    perm = np.empty(PIX, np.int64)
    perm[q * (PIX // 16) + jg] = x
    return perm


_WRAP = None


def _run(fragments, alphas, ptclds, background_color, trace=False, **kw):
    from concourse.bass_utils import run_bass_kernel_spmd

    global _WRAP
    nc = _get_nc()
    if _WRAP is None:
        _WRAP = _wrap_perm()

    table = np.ascontiguousarray(ptclds.T).astype(np.float32)  # (P, C)
    tblpad = np.zeros((NBLK, BLKF), np.float32)
    tblpad[:, 0 : RPB * C] = table.reshape(NBLK, RPB * C)
    bg4 = np.concatenate(
        [background_color.astype(np.float32), np.ones(1, np.float32)]
    ).reshape(1, C)

    in_maps = []
    for i in range(N):
        fr = np.ascontiguousarray(fragments[i].reshape(K, PIX))
        fw = fr[:, _WRAP].reshape(K, 16, PIX // 16)
        in_maps.append(
            {
                "frag": fr,
                "fragw": np.ascontiguousarray(fw),
                "alpha": np.ascontiguousarray(alphas[i].reshape(K, PIX)),
                "tbl": tblpad,
                "bg": bg4,
            }
        )

    res = run_bass_kernel_spmd(nc, in_maps, core_ids=list(range(N)), trace=trace, **kw)
    out = np.stack([res.results[i]["out"].reshape(C, H, W) for i in range(N)])
    return out.astype(np.float32), res


def kernel(fragments, alphas, ptclds, background_color):
    out, _ = _run(fragments, alphas, ptclds, background_color)
    return out
